# revision 10
# baseline (speedup 1.0000x reference)
"""Sliding-window causal self-attention (B=2, T=2048, D=1024, H=16, dk=64, W=512)
on 8 Trainium2 NeuronCores.

Sharding: core = (b, hg) for b in {0,1}, head-group hg in {0..3}.
Data parallel over batch, tensor parallel over heads: each core gets
x[b]^T, the 4-head column slices of Wq/Wk/Wv (+bq slice) and the matching
row slice of Wo, and produces a partial [T, D] output.  Host gathers with
out[b] = sum_hg partial[b,hg] + (bv @ Wo + bo).

Math notes (exact softmax identities, validated vs reference):
 - bk shifts every logit of a row by a per-row constant -> cancels in softmax.
 - bv enters the output linearly with weights summing to 1 -> folded into the
   host-side bias term bv @ Wo (+ bo), added once after the cross-core sum.
 - no max-subtraction in softmax: logits are O(1), fp32 exp is safe.

v4: bf16 operands (fp32 psum) + fine-grained PE scheduling.  The PE queue is
strict FIFO, so any matmul waiting on a psum slot blocks everything behind
it.  All projection / output-projection work is therefore chopped into
~4-matmul "filler" units and emitted BETWEEN the score matmuls of each
key-block, so the PE always has independent work while the scalar engine's
exps drain the score psum ring.  Host pre-rearranges x^T and the weights
into partition-major layouts so every input DMA is one cheap descriptor
(contiguous per partition).  All 4 heads are processed per key-block J;
per-(head,J) scores live in a [128,1024]-padded 2-bank psum slab (ring of
2) with the 640-wide band exp'd in one ACT op; triangular corner masks are
one strided DVE mul per (J, side) covering all 4 heads.
"""

import math
from contextlib import ExitStack

import numpy as np
import ml_dtypes

import concourse.bass as bass
import concourse.mybir as mybir
import concourse.tile as tile
from concourse import bacc
from concourse.bass_utils import run_bass_kernel_spmd

F32 = mybir.dt.float32
BF16 = mybir.dt.bfloat16
NPBF16 = ml_dtypes.bfloat16

T = 2048
D = 1024
NHEAD = 16
DK = 64
WINDOW = 512
HPC = 4            # heads per core
HCOLS = HPC * DK   # 256 projected columns per core
NJ = T // 128      # 16 j/query blocks
NKC = D // 128     # 8 contraction chunks over D
NG = 4             # query-block groups of 512

_NC_CACHE = {}


def _emit(tc):
    nc = tc.nc
    # partition-major host layouts: one contiguous chunk per partition
    xT_d = nc.dram_tensor("xTr", [128, 4 * NKC * 512], BF16,
                          kind="ExternalInput").ap()
    wq_d = nc.dram_tensor("wqr", [128, NKC * HCOLS], BF16,
                          kind="ExternalInput").ap()
    wk_d = nc.dram_tensor("wkr", [128, NKC * HCOLS], BF16,
                          kind="ExternalInput").ap()
    wv_d = nc.dram_tensor("wvr", [128, NKC * HCOLS], BF16,
                          kind="ExternalInput").ap()
    wo_d = nc.dram_tensor("wor", [128, 2 * D], BF16, kind="ExternalInput").ap()
    bq_d = nc.dram_tensor("bqp", [128, 2], F32, kind="ExternalInput").ap()
    msk_d = nc.dram_tensor("msk", [128, 8 * 128], BF16, kind="ExternalInput").ap()
    onv_d = nc.dram_tensor("onv", [128, NJ * HPC], BF16, kind="ExternalInput").ap()
    out_d = nc.dram_tensor("out", [T, D], BF16, kind="ExternalOutput").ap()

    with ExitStack() as ctx:
        const_pool = ctx.enter_context(tc.tile_pool(name="const", bufs=1))
        qk_pool = ctx.enter_context(tc.tile_pool(name="qk", bufs=1))
        w_pool = ctx.enter_context(tc.tile_pool(name="w", bufs=1))
        xt_pool = ctx.enter_context(tc.tile_pool(name="xt", bufs=4))
        pt_pool = ctx.enter_context(tc.tile_pool(name="pt", bufs=9))
        nrm_pool = ctx.enter_context(tc.tile_pool(name="nrm", bufs=4))
        stage_pool = ctx.enter_context(tc.tile_pool(name="stage", bufs=3))
        ps_sm = ctx.enter_context(tc.tile_pool(name="ps_sm", bufs=2, space="PSUM"))
        ps_mx = ctx.enter_context(tc.tile_pool(name="ps_mx", bufs=4, space="PSUM"))

        bq_sb = const_pool.tile([128, 2], F32)
        nc.sync.dma_start(bq_sb[:], bq_d[:, :])
        ones_row = const_pool.tile([1, 64], BF16)
        nc.vector.memset(ones_row[:], 1.0)
        # masks [128, 8, 128]: slots 0-3 = keep c >= p (x4 heads),
        # slots 4-7 = keep c < p (x4 heads)
        mask8 = const_pool.tile([128, 8, 128], BF16)
        nc.sync.dma_start(mask8[:].rearrange("p a b -> p (a b)"), msk_d[:, :])

        wo_sb = qk_pool.tile([128, 2, D], BF16)
        # V storage [j-part, J, head, dk+1]; col 64 of each head slot = 1.0
        v_sb = qk_pool.tile([128, NJ, HPC, DK + 1], BF16)
        q_sb = qk_pool.tile([128, 2, T], BF16)
        k_sb = qk_pool.tile([128, 2, T], BF16)
        osb = qk_pool.tile([128, 2, T], BF16)   # normalized O^T

        wq_sb = w_pool.tile([128, NKC, HCOLS], BF16)
        wk_sb = w_pool.tile([128, NKC, HCOLS], BF16)
        wv_sb = w_pool.tile([128, NKC, HCOLS], BF16)

        xt_tiles = {}

        def xt_dma(cb, eng0, eng1):
            xt_tiles[cb] = xt_pool.tile([128, NKC, 512], BF16, tag="xt",
                                        name=f"xt_c{cb}")
            half = NKC // 2 * 512
            base = cb * NKC * 512
            eng0.dma_start(
                xt_tiles[cb][:, 0:NKC // 2, :].rearrange("p k c -> p (k c)"),
                xT_d[:, base:base + half])
            eng1.dma_start(
                xt_tiles[cb][:, NKC // 2:NKC, :].rearrange("p k c -> p (k c)"),
                xT_d[:, base + half:base + 2 * half])

        nc.sync.dma_start(
            wq_sb[:].rearrange("p k c -> p (k c)"), wq_d[:, :])
        xt_dma(0, nc.sync, nc.gpsimd)
        nc.gpsimd.dma_start(
            wk_sb[:].rearrange("p k c -> p (k c)"), wk_d[:, :])
        xt_dma(1, nc.sync, nc.gpsimd)
        nc.gpsimd.dma_start(
            wv_sb[:].rearrange("p k c -> p (k c)"), wv_d[:, :])
        nc.sync.dma_start(
            v_sb[:, :, :, DK:DK + 1].rearrange("p j h o -> p (j h o)"),
            onv_d[:, :])
        nc.gpsimd.dma_start(
            wo_sb[:].rearrange("p c d -> p (c d)"), wo_d[:, :])
        xt_dma(2, nc.sync, nc.gpsimd)
        xt_dma(3, nc.sync, nc.gpsimd)

        # ---------- filler units: small chunks of projection work ----------
        # Each unit emits ~4 matmuls (plus psum evacuation on the last chunk)
        # so it can be slotted between dependent score matmuls.
        pend = {}

        def qk_unit(which, cb, m, half):
            w_sb = wq_sb if which == "q" else wk_sb
            key = (which, cb, m)
            nsl = slice(cb * 512, (cb + 1) * 512)
            if half == 0:
                p = ps_mx.tile([128, 512], F32, tag="mx",
                               name=f"{which}p{cb}{m}")
                pend[key] = p
            else:
                p = pend.pop(key)
            for k in range(half * 4, half * 4 + 4):
                nc.tensor.matmul(
                    p[:], w_sb[:, k, m * 128:(m + 1) * 128],
                    xt_tiles[cb][:, k, :],
                    start=(k == 0), stop=(k == NKC - 1),
                )
            if half == 1:
                if which == "q":
                    nc.scalar.activation(
                        q_sb[:, m, nsl], p[:],
                        mybir.ActivationFunctionType.Identity,
                        bias=bq_sb[:, m:m + 1],
                    )
                else:
                    nc.vector.tensor_copy(k_sb[:, m, nsl], p[:])

        def v_unit(r, half):
            cb = r // 4
            key = ("v", r)
            if half == 0:
                p = ps_mx.tile([128, HPC, DK], F32, tag="mx", name=f"vp{r}")
                pend[key] = p
            else:
                p = pend.pop(key)
            for k in range(half * 4, half * 4 + 4):
                nc.tensor.matmul(
                    p[:], xt_tiles[cb][:, k, (r % 4) * 128:(r % 4) * 128 + 128],
                    wv_sb[:, k, :], start=(k == 0), stop=(k == NKC - 1),
                )
            if half == 1:
                nc.vector.tensor_copy(v_sb[:, r, :, 0:DK], p[:])

        def oproj_unit(qb):
            so = stage_pool.tile([128, 1024], BF16, tag="stage",
                                 name=f"so{qb}")
            for nh in range(2):
                po = ps_mx.tile([128, 512], F32, tag="mx",
                                name=f"po{qb}_{nh}")
                for c in range(2):
                    nc.tensor.matmul(
                        po[:], osb[:, c, qb * 128:(qb + 1) * 128],
                        wo_sb[:, c, nh * 512:(nh + 1) * 512],
                        start=(c == 0), stop=(c == 1),
                    )
                if nh == 0:
                    nc.scalar.copy(so[:, 0:512], po[:])
                else:
                    nc.vector.tensor_copy(so[:, 512:1024], po[:])
            nc.sync.dma_start(out_d[qb * 128:(qb + 1) * 128, :], so[:, :])

        def mk_fillers():
            fills = []
            for cb in (2, 3):
                for m in (0, 1):
                    for which in ("q", "k"):
                        for half in (0, 1):
                            fills.append(
                                lambda w=which, c=cb, mm=m, h=half:
                                qk_unit(w, c, mm, h))
            return fills

        qk_fillers = mk_fillers()     # 16 units: qk2 first 8, qk3 next 8
        v_fillers = [lambda r=r, h=h: v_unit(r, h)
                     for r in range(4, 16) for h in (0, 1)]   # 24 units

        # per-J consumption: deadlines: qk2 (units 0-7) before J=4, qk3
        # (8-15) before J=8; v1 (units 0-7) before J=7, v2 (8-15) before
        # J=11, v3 (16-23) before J=15.
        fill_plan = {0: (3, 1), 1: (3, 1), 2: (2, 2), 3: (0, 2),
                     4: (3, 1), 5: (3, 1), 6: (2, 2), 7: (0, 2),
                     8: (0, 3), 9: (0, 3), 10: (0, 2), 11: (0, 2),
                     12: (0, 2), 13: (0, 0), 14: (0, 0), 15: (0, 0)}

        def fill(nq, nv):
            for _ in range(nq):
                if qk_fillers:
                    qk_fillers.pop(0)()
            for _ in range(nv):
                if v_fillers:
                    v_fillers.pop(0)()

        def scores_pair(pt4, J, hh, width, wA, wB):
            sms = []
            for h in (2 * hh, 2 * hh + 1):   # concurrent PE row-tiles
                hp = slice((h % 2) * 64, (h % 2) * 64 + 64)
                sm = ps_sm.tile([128, 1024], F32, tag="sm",
                                name=f"sm_h{h}_J{J}")
                sms.append((h, hp, sm))
                nc.tensor.matmul(
                    sm[:, 0:wA], k_sb[hp, hh, J * 128:(J + 1) * 128],
                    q_sb[hp, hh, J * 128:J * 128 + wA],
                    start=True, stop=True,
                )
            if wB > 0:
                for h, hp, sm in sms:
                    nc.tensor.matmul(
                        sm[:, 512:512 + wB],
                        k_sb[hp, hh, J * 128:(J + 1) * 128],
                        q_sb[hp, hh, J * 128 + 512:J * 128 + width],
                        start=True, stop=True,
                    )
            for h, hp, sm in sms:
                nc.scalar.activation(
                    pt4[:, h, 0:width], sm[:, 0:width],
                    mybir.ActivationFunctionType.Exp, scale=0.125,
                )

        def masks_j(pt4, wB):
            if wB > 0:
                nc.vector.tensor_mul(
                    pt4[:, :, 512:512 + wB], pt4[:, :, 512:512 + wB],
                    mask8[:, 4:8, 0:wB])
            nc.gpsimd.tensor_mul(
                pt4[:, :, 0:128], pt4[:, :, 0:128], mask8[:, 0:4, :])

        def attn_group(pt_tiles, g):
            """PV accumulation + normalization for all heads of group g."""
            g0 = 512 * g
            jps = []
            for Jp in range(max(0, 4 * g - 4), 4 * g + 4):
                wJp = min(640, T - Jp * 128)
                lo = max(Jp * 128, g0)
                hi = min(Jp * 128 + wJp, g0 + 512)
                if hi > lo:
                    jps.append((Jp, lo, hi))
            # start=True lazily zeroes the whole psum bank; a full-width
            # contribution must come first
            jps.sort(key=lambda t: -(t[2] - t[1]))
            assert jps[0][2] - jps[0][1] == 512
            for h in range(HPC):
                hp = slice((h % 2) * 64, (h % 2) * 64 + 64)
                hc = h // 2
                pv = ps_mx.tile([65, 512], F32, tag="mx", name=f"pv_h{h}_g{g}")
                for idx, (Jp, lo, hi) in enumerate(jps):
                    nc.tensor.matmul(
                        pv[:, lo - g0:hi - g0],
                        v_sb[:, Jp, h, :],
                        pt_tiles[Jp][:, h, lo - Jp * 128:hi - Jp * 128],
                        start=(idx == 0), stop=(idx == len(jps) - 1),
                    )
                den = nrm_pool.tile([1, 512], BF16, tag="den",
                                    name=f"den_h{h}_g{g}")
                nc.vector.tensor_copy(den[:], pv[64:65, :])
                bcp = ps_mx.tile([64, 512], F32, tag="mx", name=f"bcp_h{h}_g{g}")
                nc.tensor.matmul(bcp[:], ones_row[:], den[:],
                                 start=True, stop=True)
                rcb = nrm_pool.tile([64, 512], F32, tag="rcb",
                                    name=f"rcb_h{h}_g{g}")
                nc.vector.reciprocal_approx_fast(rcb[:], bcp[:])
                nc.vector.tensor_mul(
                    osb[hp, hc, g0:g0 + 512], pv[0:64, :], rcb[:],
                )
            for Jp in range(max(0, 4 * g - 4), 4 * g):
                pt_tiles.pop(Jp, None)

        # ---- schedule: dense-PE J loop with filler interleaving ----
        for cb in (0, 1):
            for m in (0, 1):
                for which in ("q", "k"):
                    for half in (0, 1):
                        qk_unit(which, cb, m, half)
        for r in range(0, 4):
            v_unit(r, 0)
            v_unit(r, 1)

        pts = {}
        for J in range(NJ):
            width = min(640, T - J * 128)
            wA = min(512, width)
            wB = width - wA
            pt4 = pt_pool.tile([128, HPC, 640], BF16, tag="pt",
                               name=f"pt4_J{J}")
            pts[J] = pt4
            nq, nv = fill_plan[J]
            scores_pair(pt4, J, 0, width, wA, wB)
            if J % 4 == 0 and J >= 4:
                oproj_unit(4 * (J // 4 - 1) + 0)
                oproj_unit(4 * (J // 4 - 1) + 1)
            fill(nq - nq // 2, nv - nv // 2)
            scores_pair(pt4, J, 1, width, wA, wB)
            masks_j(pt4, wB)
            if J % 4 == 0 and J >= 4:
                oproj_unit(4 * (J // 4 - 1) + 2)
                oproj_unit(4 * (J // 4 - 1) + 3)
            fill(nq // 2, nv // 2)
            if J % 4 == 3:
                attn_group(pts, J // 4)
        for qb in range(4 * (NG - 1), 4 * NG):
            oproj_unit(qb)


def _build():
    if "nc" in _NC_CACHE:
        return _NC_CACHE["nc"]
    nc = bacc.Bacc("TRN2", debug=False)
    with tile.TileContext(nc) as tc:
        _emit(tc)
    nc.compile()
    _NC_CACHE["nc"] = nc
    return nc


def _pmajor(w, nchunk):
    """[nchunk*128, C] -> [128, nchunk*C] partition-major bf16."""
    c = w.shape[1]
    return np.ascontiguousarray(
        w.reshape(nchunk, 128, c).transpose(1, 0, 2).reshape(128, nchunk * c)
    ).astype(NPBF16)


def _shard_inputs(x, Wq, bq, Wk, Wv, Wo):
    idx = np.arange(128)
    mlo = (idx[None, :] >= idx[:, None]).astype(NPBF16)  # c >= p
    mhi = (idx[None, :] < idx[:, None]).astype(NPBF16)   # c < p
    msk = np.concatenate([mlo] * 4 + [mhi] * 4, axis=1)  # [128, 1024]
    onv = np.ones((128, NJ * HPC), NPBF16)
    in_maps = []
    for b in range(2):
        xT = np.ascontiguousarray(x[b].T)                # [D, T]
        # [128, cb, k, 512] partition-major: per-partition contiguous chunks
        xTr = np.ascontiguousarray(
            xT.reshape(NKC, 128, 4, 512).transpose(1, 2, 0, 3)
            .reshape(128, 4 * NKC * 512)).astype(NPBF16)
        for hg in range(4):
            cols = slice(hg * HCOLS, (hg + 1) * HCOLS)
            in_maps.append({
                "xTr": xTr,
                "wqr": _pmajor(Wq[:, cols], NKC),
                "wkr": _pmajor(Wk[:, cols], NKC),
                "wvr": _pmajor(Wv[:, cols], NKC),
                "wor": _pmajor(Wo[cols, :], 2),
                "bqp": np.ascontiguousarray(bq[cols].reshape(2, 128).T),
                "msk": msk, "onv": onv,
            })
    return in_maps


def kernel(x, Wq, bq, Wk, bk, Wv, bv, Wo, bo, _trace=False, _tmpdir=None):
    x = np.asarray(x, dtype=np.float32)
    Wq = np.asarray(Wq, dtype=np.float32)
    Wk = np.asarray(Wk, dtype=np.float32)
    Wv = np.asarray(Wv, dtype=np.float32)
    Wo = np.asarray(Wo, dtype=np.float32)
    bq = np.asarray(bq, dtype=np.float32)
    bv = np.asarray(bv, dtype=np.float32)
    bo = np.asarray(bo, dtype=np.float32)

    nc = _build()
    in_maps = _shard_inputs(x, Wq, bq, Wk, Wv, Wo)
    res = run_bass_kernel_spmd(
        nc, in_maps, core_ids=list(range(8)), trace=_trace, tmpdir=_tmpdir,
    )
    host_bias = (bv @ Wo + bo).astype(np.float32)
    out = np.zeros((2, T, D), dtype=np.float32)
    for b in range(2):
        acc = res.results[b * 4]["out"].astype(np.float32)
        for hg in range(1, 4):
            acc = acc + res.results[b * 4 + hg]["out"].astype(np.float32)
        out[b] = acc + host_bias
    kernel._last_results = res
    return out


# revision 11
# speedup vs baseline: 1.0960x; 1.0960x over previous
"""Sliding-window causal self-attention (B=2, T=2048, D=1024, H=16, dk=64, W=512)
on 8 Trainium2 NeuronCores.

Sharding: core = (b, hg) for b in {0,1}, head-group hg in {0..3}.
Data parallel over batch, tensor parallel over heads: each core gets
x[b]^T, the 4-head column slices of Wq/Wk/Wv (+bq slice) and the matching
row slice of Wo, and produces a partial [T, D] output.  Host gathers with
out[b] = sum_hg partial[b,hg] + (bv @ Wo + bo).

Math notes (exact softmax identities, validated vs reference):
 - bk shifts every logit of a row by a per-row constant -> cancels in softmax.
 - bv enters the output linearly with weights summing to 1 -> folded into the
   host-side bias term bv @ Wo (+ bo), added once after the cross-core sum.
 - no max-subtraction in softmax: logits are O(1), fp32 exp is safe.

v4: bf16 operands (fp32 psum) + fine-grained PE scheduling.  The PE queue is
strict FIFO, so any matmul waiting on a psum slot blocks everything behind
it.  All projection / output-projection work is therefore chopped into
~4-matmul "filler" units and emitted BETWEEN the score matmuls of each
key-block, so the PE always has independent work while the scalar engine's
exps drain the score psum ring.  Host pre-rearranges x^T and the weights
into partition-major layouts so every input DMA is one cheap descriptor
(contiguous per partition).  All 4 heads are processed per key-block J;
per-(head,J) scores live in a [128,1024]-padded 2-bank psum slab (ring of
2) with the 640-wide band exp'd in one ACT op; triangular corner masks are
one strided DVE mul per (J, side) covering all 4 heads.
"""

import math
from contextlib import ExitStack

import numpy as np
import ml_dtypes

import concourse.bass as bass
import concourse.mybir as mybir
import concourse.tile as tile
from concourse import bacc
from concourse.bass_utils import run_bass_kernel_spmd

F32 = mybir.dt.float32
BF16 = mybir.dt.bfloat16
NPBF16 = ml_dtypes.bfloat16

T = 2048
D = 1024
NHEAD = 16
DK = 64
WINDOW = 512
HPC = 4            # heads per core
HCOLS = HPC * DK   # 256 projected columns per core
NJ = T // 128      # 16 j/query blocks
NKC = D // 128     # 8 contraction chunks over D
NG = 4             # query-block groups of 512

_NC_CACHE = {}


def _emit(tc):
    nc = tc.nc
    # partition-major host layouts: one contiguous chunk per partition
    xT_d = nc.dram_tensor("xTr", [128, 4 * NKC * 512], BF16,
                          kind="ExternalInput").ap()
    wq_d = nc.dram_tensor("wqr", [128, NKC * HCOLS], BF16,
                          kind="ExternalInput").ap()
    wk_d = nc.dram_tensor("wkr", [128, NKC * HCOLS], BF16,
                          kind="ExternalInput").ap()
    wv_d = nc.dram_tensor("wvr", [128, NKC * HCOLS], BF16,
                          kind="ExternalInput").ap()
    wo_d = nc.dram_tensor("wor", [128, 2 * D], BF16, kind="ExternalInput").ap()
    bq_d = nc.dram_tensor("bqp", [128, 2], F32, kind="ExternalInput").ap()
    msk_d = nc.dram_tensor("msk", [128, 8 * 128], BF16, kind="ExternalInput").ap()
    onv_d = nc.dram_tensor("onv", [128, NJ * HPC], BF16, kind="ExternalInput").ap()
    out_d = nc.dram_tensor("out", [T, D], BF16, kind="ExternalOutput").ap()

    with ExitStack() as ctx:
        const_pool = ctx.enter_context(tc.tile_pool(name="const", bufs=1))
        qk_pool = ctx.enter_context(tc.tile_pool(name="qk", bufs=1))
        w_pool = ctx.enter_context(tc.tile_pool(name="w", bufs=1))
        xt_pool = ctx.enter_context(tc.tile_pool(name="xt", bufs=4))
        pt_pool = ctx.enter_context(tc.tile_pool(name="pt", bufs=9))
        nrm_pool = ctx.enter_context(tc.tile_pool(name="nrm", bufs=4))
        stage_pool = ctx.enter_context(tc.tile_pool(name="stage", bufs=3))
        ps_sm = ctx.enter_context(tc.tile_pool(name="ps_sm", bufs=2, space="PSUM"))
        ps_mx = ctx.enter_context(tc.tile_pool(name="ps_mx", bufs=4, space="PSUM"))

        bq_sb = const_pool.tile([128, 2], F32)
        nc.sync.dma_start(bq_sb[:], bq_d[:, :])
        ones_row = const_pool.tile([1, 64], BF16)
        nc.vector.memset(ones_row[:], 1.0)
        # masks [128, 8, 128]: slots 0-3 = keep c >= p (x4 heads),
        # slots 4-7 = keep c < p (x4 heads)
        mask8 = const_pool.tile([128, 8, 128], BF16)
        nc.sync.dma_start(mask8[:].rearrange("p a b -> p (a b)"), msk_d[:, :])

        wo_sb = qk_pool.tile([128, 2, D], BF16)
        # V storage [j-part, J, head, dk+1]; col 64 of each head slot = 1.0
        v_sb = qk_pool.tile([128, NJ, HPC, DK + 1], BF16)
        q_sb = qk_pool.tile([128, 2, T], BF16)
        k_sb = qk_pool.tile([128, 2, T], BF16)
        osb = qk_pool.tile([128, 2, T], BF16)   # normalized O^T

        wq_sb = w_pool.tile([128, NKC, HCOLS], BF16)
        wk_sb = w_pool.tile([128, NKC, HCOLS], BF16)
        wv_sb = w_pool.tile([128, NKC, HCOLS], BF16)

        xt_tiles = {}

        def xt_dma(cb, eng0, eng1):
            xt_tiles[cb] = xt_pool.tile([128, NKC, 512], BF16, tag="xt",
                                        name=f"xt_c{cb}")
            half = NKC // 2 * 512
            base = cb * NKC * 512
            eng0.dma_start(
                xt_tiles[cb][:, 0:NKC // 2, :].rearrange("p k c -> p (k c)"),
                xT_d[:, base:base + half])
            eng1.dma_start(
                xt_tiles[cb][:, NKC // 2:NKC, :].rearrange("p k c -> p (k c)"),
                xT_d[:, base + half:base + 2 * half])

        nc.sync.dma_start(
            wq_sb[:].rearrange("p k c -> p (k c)"), wq_d[:, :])
        xt_dma(0, nc.sync, nc.gpsimd)
        nc.gpsimd.dma_start(
            wk_sb[:].rearrange("p k c -> p (k c)"), wk_d[:, :])
        xt_dma(1, nc.sync, nc.gpsimd)
        nc.gpsimd.dma_start(
            wv_sb[:].rearrange("p k c -> p (k c)"), wv_d[:, :])
        nc.sync.dma_start(
            v_sb[:, :, :, DK:DK + 1].rearrange("p j h o -> p (j h o)"),
            onv_d[:, :])
        nc.gpsimd.dma_start(
            wo_sb[:].rearrange("p c d -> p (c d)"), wo_d[:, :])
        xt_dma(2, nc.sync, nc.gpsimd)
        xt_dma(3, nc.sync, nc.gpsimd)

        # ---------- filler units: small chunks of projection work ----------
        # Each unit emits ~4 matmuls (plus psum evacuation on the last chunk)
        # so it can be slotted between dependent score matmuls.
        pend = {}

        def qk_unit(which, cb, m, half):
            w_sb = wq_sb if which == "q" else wk_sb
            key = (which, cb, m)
            nsl = slice(cb * 512, (cb + 1) * 512)
            if half == 0:
                p = ps_mx.tile([128, 512], F32, tag="mx",
                               name=f"{which}p{cb}{m}")
                pend[key] = p
            else:
                p = pend.pop(key)
            for k in range(half * 4, half * 4 + 4):
                nc.tensor.matmul(
                    p[:], w_sb[:, k, m * 128:(m + 1) * 128],
                    xt_tiles[cb][:, k, :],
                    start=(k == 0), stop=(k == NKC - 1),
                )
            if half == 1:
                if which == "q":
                    nc.scalar.activation(
                        q_sb[:, m, nsl], p[:],
                        mybir.ActivationFunctionType.Identity,
                        bias=bq_sb[:, m:m + 1],
                    )
                else:
                    nc.vector.tensor_copy(k_sb[:, m, nsl], p[:])

        def v_unit(r, half):
            cb = r // 4
            key = ("v", r)
            if half == 0:
                p = ps_mx.tile([128, HPC, DK], F32, tag="mx", name=f"vp{r}")
                pend[key] = p
            else:
                p = pend.pop(key)
            for k in range(half * 4, half * 4 + 4):
                nc.tensor.matmul(
                    p[:], xt_tiles[cb][:, k, (r % 4) * 128:(r % 4) * 128 + 128],
                    wv_sb[:, k, :], start=(k == 0), stop=(k == NKC - 1),
                )
            if half == 1:
                nc.vector.tensor_copy(v_sb[:, r, :, 0:DK], p[:])

        def oproj_unit(qb):
            so = stage_pool.tile([128, 1024], BF16, tag="stage",
                                 name=f"so{qb}")
            for nh in range(2):
                po = ps_mx.tile([128, 512], F32, tag="mx",
                                name=f"po{qb}_{nh}")
                for c in range(2):
                    nc.tensor.matmul(
                        po[:], osb[:, c, qb * 128:(qb + 1) * 128],
                        wo_sb[:, c, nh * 512:(nh + 1) * 512],
                        start=(c == 0), stop=(c == 1),
                    )
                if nh == 0:
                    nc.scalar.copy(so[:, 0:512], po[:])
                else:
                    nc.vector.tensor_copy(so[:, 512:1024], po[:])
            nc.sync.dma_start(out_d[qb * 128:(qb + 1) * 128, :], so[:, :])

        def mk_fillers():
            fills = []
            for cb in (2, 3):
                for m in (0, 1):
                    for which in ("q", "k"):
                        for half in (0, 1):
                            fills.append(
                                lambda w=which, c=cb, mm=m, h=half:
                                qk_unit(w, c, mm, h))
            return fills

        qk_fillers = mk_fillers()     # 16 units: qk2 first 8, qk3 next 8
        v_fillers = [lambda r=r, h=h: v_unit(r, h)
                     for r in range(4, 16) for h in (0, 1)]   # 24 units

        # per-J consumption: deadlines: qk2 (units 0-7) before J=4, qk3
        # (8-15) before J=8; v1 (units 0-7) before J=7, v2 (8-15) before
        # J=11, v3 (16-23) before J=15.  Spread evenly so the PE has
        # independent work in EVERY HAM window, including the tail.
        fill_plan = {0: (2, 0), 1: (2, 1), 2: (2, 1), 3: (2, 2),
                     4: (2, 1), 5: (2, 1), 6: (2, 2), 7: (2, 2),
                     8: (0, 2), 9: (0, 2), 10: (0, 2), 11: (0, 2),
                     12: (0, 2), 13: (0, 2), 14: (0, 2), 15: (0, 0)}

        def fill(nq, nv):
            for _ in range(nq):
                if qk_fillers:
                    qk_fillers.pop(0)()
            for _ in range(nv):
                if v_fillers:
                    v_fillers.pop(0)()

        def scores_pair(pt4, J, hh, width, wA, wB):
            sms = []
            for h in (2 * hh, 2 * hh + 1):   # concurrent PE row-tiles
                hp = slice((h % 2) * 64, (h % 2) * 64 + 64)
                sm = ps_sm.tile([128, 1024], F32, tag="sm",
                                name=f"sm_h{h}_J{J}")
                sms.append((h, hp, sm))
                nc.tensor.matmul(
                    sm[:, 0:wA], k_sb[hp, hh, J * 128:(J + 1) * 128],
                    q_sb[hp, hh, J * 128:J * 128 + wA],
                    start=True, stop=True,
                )
            if wB > 0:
                for h, hp, sm in sms:
                    nc.tensor.matmul(
                        sm[:, 512:512 + wB],
                        k_sb[hp, hh, J * 128:(J + 1) * 128],
                        q_sb[hp, hh, J * 128 + 512:J * 128 + width],
                        start=True, stop=True,
                    )
            for h, hp, sm in sms:
                nc.scalar.activation(
                    pt4[:, h, 0:width], sm[:, 0:width],
                    mybir.ActivationFunctionType.Exp, scale=0.125,
                )

        def masks_j(pt4, wB, on_vector=False):
            if wB > 0:
                nc.vector.tensor_mul(
                    pt4[:, :, 512:512 + wB], pt4[:, :, 512:512 + wB],
                    mask8[:, 4:8, 0:wB])
            eng = nc.vector if on_vector else nc.gpsimd
            eng.tensor_mul(
                pt4[:, :, 0:128], pt4[:, :, 0:128], mask8[:, 0:4, :])

        def attn_group(pt_tiles, g):
            """PV accumulation + normalization for all heads of group g."""
            g0 = 512 * g
            jps = []
            for Jp in range(max(0, 4 * g - 4), 4 * g + 4):
                wJp = min(640, T - Jp * 128)
                lo = max(Jp * 128, g0)
                hi = min(Jp * 128 + wJp, g0 + 512)
                if hi > lo:
                    jps.append((Jp, lo, hi))
            # start=True lazily zeroes the whole psum bank; a full-width
            # contribution must come first
            jps.sort(key=lambda t: -(t[2] - t[1]))
            assert jps[0][2] - jps[0][1] == 512
            for h in range(HPC):
                hp = slice((h % 2) * 64, (h % 2) * 64 + 64)
                hc = h // 2
                pv = ps_mx.tile([65, 512], F32, tag="mx", name=f"pv_h{h}_g{g}")
                for idx, (Jp, lo, hi) in enumerate(jps):
                    nc.tensor.matmul(
                        pv[:, lo - g0:hi - g0],
                        v_sb[:, Jp, h, :],
                        pt_tiles[Jp][:, h, lo - Jp * 128:hi - Jp * 128],
                        start=(idx == 0), stop=(idx == len(jps) - 1),
                    )
                den = nrm_pool.tile([1, 512], BF16, tag="den",
                                    name=f"den_h{h}_g{g}")
                nc.vector.tensor_copy(den[:], pv[64:65, :])
                bcp = ps_mx.tile([64, 512], F32, tag="mx", name=f"bcp_h{h}_g{g}")
                nc.tensor.matmul(bcp[:], ones_row[:], den[:],
                                 start=True, stop=True)
                rcb = nrm_pool.tile([64, 512], F32, tag="rcb",
                                    name=f"rcb_h{h}_g{g}")
                nc.vector.reciprocal_approx_fast(rcb[:], bcp[:])
                nc.vector.tensor_mul(
                    osb[hp, hc, g0:g0 + 512], pv[0:64, :], rcb[:],
                )
            for Jp in range(max(0, 4 * g - 4), 4 * g):
                pt_tiles.pop(Jp, None)

        # ---- schedule: dense-PE J loop with filler interleaving ----
        for cb in (0, 1):
            for m in (0, 1):
                for which in ("q", "k"):
                    for half in (0, 1):
                        qk_unit(which, cb, m, half)
        for r in range(0, 4):
            v_unit(r, 0)
            v_unit(r, 1)

        pts = {}
        for J in range(NJ):
            width = min(640, T - J * 128)
            wA = min(512, width)
            wB = width - wA
            pt4 = pt_pool.tile([128, HPC, 640], BF16, tag="pt",
                               name=f"pt4_J{J}")
            pts[J] = pt4
            nq, nv = fill_plan[J]
            scores_pair(pt4, J, 0, width, wA, wB)
            if J >= 4:
                # output projection of group (J-4)//4, one block per J
                oproj_unit(J - 4)
            fill(nq - nq // 2, nv - nv // 2)
            scores_pair(pt4, J, 1, width, wA, wB)
            masks_j(pt4, wB, on_vector=(J % 4 == 3))
            fill(nq // 2, nv // 2)
            if J % 4 == 3:
                attn_group(pts, J // 4)
        for qb in range(12, 16):
            oproj_unit(qb)


def _build():
    if "nc" in _NC_CACHE:
        return _NC_CACHE["nc"]
    nc = bacc.Bacc("TRN2", debug=False)
    with tile.TileContext(nc) as tc:
        _emit(tc)
    nc.compile()
    _NC_CACHE["nc"] = nc
    return nc


def _pmajor(w, nchunk):
    """[nchunk*128, C] -> [128, nchunk*C] partition-major bf16."""
    c = w.shape[1]
    return np.ascontiguousarray(
        w.reshape(nchunk, 128, c).transpose(1, 0, 2).reshape(128, nchunk * c)
    ).astype(NPBF16)


def _shard_inputs(x, Wq, bq, Wk, Wv, Wo):
    idx = np.arange(128)
    mlo = (idx[None, :] >= idx[:, None]).astype(NPBF16)  # c >= p
    mhi = (idx[None, :] < idx[:, None]).astype(NPBF16)   # c < p
    msk = np.concatenate([mlo] * 4 + [mhi] * 4, axis=1)  # [128, 1024]
    onv = np.ones((128, NJ * HPC), NPBF16)
    in_maps = []
    for b in range(2):
        xT = np.ascontiguousarray(x[b].T)                # [D, T]
        # [128, cb, k, 512] partition-major: per-partition contiguous chunks
        xTr = np.ascontiguousarray(
            xT.reshape(NKC, 128, 4, 512).transpose(1, 2, 0, 3)
            .reshape(128, 4 * NKC * 512)).astype(NPBF16)
        for hg in range(4):
            cols = slice(hg * HCOLS, (hg + 1) * HCOLS)
            in_maps.append({
                "xTr": xTr,
                "wqr": _pmajor(Wq[:, cols], NKC),
                "wkr": _pmajor(Wk[:, cols], NKC),
                "wvr": _pmajor(Wv[:, cols], NKC),
                "wor": _pmajor(Wo[cols, :], 2),
                "bqp": np.ascontiguousarray(bq[cols].reshape(2, 128).T),
                "msk": msk, "onv": onv,
            })
    return in_maps


def kernel(x, Wq, bq, Wk, bk, Wv, bv, Wo, bo, _trace=False, _tmpdir=None):
    x = np.asarray(x, dtype=np.float32)
    Wq = np.asarray(Wq, dtype=np.float32)
    Wk = np.asarray(Wk, dtype=np.float32)
    Wv = np.asarray(Wv, dtype=np.float32)
    Wo = np.asarray(Wo, dtype=np.float32)
    bq = np.asarray(bq, dtype=np.float32)
    bv = np.asarray(bv, dtype=np.float32)
    bo = np.asarray(bo, dtype=np.float32)

    nc = _build()
    in_maps = _shard_inputs(x, Wq, bq, Wk, Wv, Wo)
    res = run_bass_kernel_spmd(
        nc, in_maps, core_ids=list(range(8)), trace=_trace, tmpdir=_tmpdir,
    )
    host_bias = (bv @ Wo + bo).astype(np.float32)
    out = np.zeros((2, T, D), dtype=np.float32)
    for b in range(2):
        acc = res.results[b * 4]["out"].astype(np.float32)
        for hg in range(1, 4):
            acc = acc + res.results[b * 4 + hg]["out"].astype(np.float32)
        out[b] = acc + host_bias
    kernel._last_results = res
    return out
